# revision 1
# baseline (speedup 1.0000x reference)
"""Distributed sparse-attention kernel for one TRN2 chip (8 NeuronCores).

Sharding: sequence-parallel. Core c owns queries [256*c, 256*(c+1)) for all
8 heads. Each core computes K/V for the full sequence (needed because the
fine branch may select any past block), runs all three attention branches
for its own query slice, applies gating and the final output projection for
its own rows. No cross-core collective is needed; the host concatenates the
eight row-slices.
"""

import numpy as np
import jax
import jax.numpy as jnp

DIM = 512; H = 8; DH = 64
WIN = 64; CBS = 16; SBS = 16; NSEL = 4
B = 1; N = 2048
NCORES = 8
QS = N // NCORES          # 256 queries per core
NB = N // CBS             # 128 blocks
SCALE = DH ** -0.5


def _shard_fn(x, Wq, Wk, Wv, Wo, Wg, Wck, Wcv, k_pos, v_pos, mem_k, mem_v, off):
    """Compute output rows [off, off+QS) given the full input x."""
    # full-sequence K/V (every core needs all past keys/values)
    k = (x @ Wk).reshape(N, H, DH).transpose(1, 0, 2)          # [H, N, DH]
    v = (x @ Wv).reshape(N, H, DH).transpose(1, 0, 2)
    xq = jax.lax.dynamic_slice_in_dim(x, off, QS, axis=0)      # [QS, DIM]
    q = (xq @ Wq).reshape(QS, H, DH).transpose(1, 0, 2)        # [H, QS, DH]
    qpos = off + jnp.arange(QS)                                # [QS]

    # ---- branch 1: compressed attention ----
    kb = k.reshape(H, NB, CBS, DH) + k_pos[:, None]            # [H, NB, CBS, DH]
    vb = v.reshape(H, NB, CBS, DH) + v_pos[:, None]
    ck = kb.reshape(H, NB, CBS * DH) @ Wck                     # [H, NB, DH]
    cv = vb.reshape(H, NB, CBS * DH) @ Wcv
    ck_all = jnp.concatenate([mem_k, ck], 1)                   # [H, NB+1, DH]
    cv_all = jnp.concatenate([mem_v, cv], 1)
    csim = jnp.einsum('hnd,hjd->hnj', q, ck_all) * SCALE       # [H, QS, NB+1]
    cmask_blk = qpos[:, None] >= (jnp.arange(NB) + 1) * CBS    # [QS, NB]
    cmask = jnp.concatenate([jnp.ones((QS, 1), bool), cmask_blk], 1)
    cattn = jax.nn.softmax(jnp.where(cmask[None], csim, -jnp.inf), -1)
    cout = jnp.einsum('hnj,hjd->hnd', cattn, cv_all)           # [H, QS, DH]

    # ---- branch 2: fine attention over top-k selected blocks ----
    # Dense-masked formulation: instead of gathering the selected K/V blocks
    # per query (gather-heavy), compute the full q@k^T row and mask it down
    # to the selected blocks. Ties at zero importance are all fully-future
    # blocks, which the causal mask zeroes either way, so a threshold mask
    # reproduces top_k exactly.
    imp = cattn[..., 1:]                                       # [H, QS, NB]
    own = (qpos // SBS)[:, None] == jnp.arange(NB)[None]       # [QS, NB]
    imp = jnp.where(own[None], 1e9, imp)
    _, sel = jax.lax.top_k(imp, NSEL)                          # [H, QS, NSEL]
    selmask = (sel[..., None] == jnp.arange(NB)).any(-2)       # [H, QS, NB]
    S = jnp.einsum('hnd,hkd->hnk', q, k) * SCALE               # [H, QS, N]
    causal = qpos[:, None] >= jnp.arange(N)[None]              # [QS, N]
    fmask = jnp.repeat(selmask, SBS, axis=-1) & causal[None]   # [H, QS, N]
    fattn = jax.nn.softmax(jnp.where(fmask, S, -jnp.inf), -1)
    fout = jnp.einsum('hnk,hkd->hnd', fattn, v)                # [H, QS, DH]

    # ---- branch 3: causal sliding-window attention (banded slice) ----
    # Keys for queries [off, off+QS) lie in [off-WIN+1, off+QS); slice a
    # QS+WIN key window (clamped at 0 for core 0; masked out anyway).
    koff = jnp.maximum(off - WIN, 0)
    kwin = jax.lax.dynamic_slice_in_dim(k, koff, QS + WIN, 1)  # [H, QS+WIN, DH]
    vwin = jax.lax.dynamic_slice_in_dim(v, koff, QS + WIN, 1)
    kwpos = koff + jnp.arange(QS + WIN)                        # [QS+WIN]
    d = qpos[:, None] - kwpos[None]                            # [QS, QS+WIN]
    wvalid = (d >= 0) & (d < WIN)
    wsim = jnp.einsum('hnd,hkd->hnk', q, kwin) * SCALE
    wattn = jax.nn.softmax(jnp.where(wvalid[None], wsim, -jnp.inf), -1)
    wout = jnp.einsum('hnk,hkd->hnd', wattn, vwin)             # [H, QS, DH]

    # ---- gating + output projection (own rows only) ----
    g = jax.nn.sigmoid(xq @ Wg).reshape(QS, 3, H)              # [QS, 3, H]
    g = g.transpose(1, 2, 0)[..., None]                        # [3, H, QS, 1]
    out = g[0] * cout + g[1] * fout + g[2] * wout              # [H, QS, DH]
    return (out.transpose(1, 0, 2).reshape(QS, H * DH)) @ Wo   # [QS, DIM]


_pmapped = jax.pmap(
    _shard_fn,
    in_axes=(None, None, None, None, None, None, None, None, None, None,
             None, None, 0),
    devices=jax.devices()[:NCORES],
)


def kernel(**inputs):
    x = np.asarray(inputs['x']).reshape(N, DIM)
    offs = np.arange(NCORES, dtype=np.int32) * QS
    out = _pmapped(
        jnp.asarray(x),
        jnp.asarray(inputs['Wq']), jnp.asarray(inputs['Wk']),
        jnp.asarray(inputs['Wv']), jnp.asarray(inputs['Wo']),
        jnp.asarray(inputs['Wg']), jnp.asarray(inputs['Wck']),
        jnp.asarray(inputs['Wcv']), jnp.asarray(inputs['k_pos']),
        jnp.asarray(inputs['v_pos']), jnp.asarray(inputs['mem_k']),
        jnp.asarray(inputs['mem_v']), jnp.asarray(offs),
    )                                                          # [8, QS, DIM]
    return np.asarray(out).reshape(B, N, DIM).astype(np.float32)

